# revision 16
# baseline (speedup 1.0000x reference)
"""Checksum-based fault detection + correction for C = B @ A.T on 8 trn2 cores.

Full inputs in, full output out. Rows of B / C_faulty are sharded across the
8 cores (data-parallel row slabs); A is replicated. C is streamed through the
device in fp16 (host casts during shard/gather; the harness gate is rel_err,
and fp16 round-trip costs ~2e-4 while halving HBM traffic).

Per 128-row slab x 4096-col group, each core:
  - streams the C group in (SP-triggered DMA, 1 MB per transfer),
  - ACT engine copies it to the output staging buffer obuf with a fused
    accumulate that yields the per-row group sum sc = sum_cols(C[p, group])
    in the same pass (detection coupling to C costs no extra engine pass),
  - PE computes the expected group sums chk = B_slab @ s_g (s_g = column
    sums of A.T per group, host-precomputed, one tiny 2-col matmul per slab)
    and recomputes ct = B_slab @ A.T chunk by chunk (8 x 512-col matmuls),
  - DVE forms the row flag m[p] = ((chk - sc) < -50) as uint16 in one small
    tensor_scalar op (faults shift a group sum by ~ +100 per faulty element;
    numerical noise is < ~4), then patches flagged rows of obuf from the
    recompute PSUM with one copy_predicated per 512-col chunk,
  - streams obuf back out (SP-triggered DMA).

Engine budget per core (16 groups): DMA ~92us (bound), DVE ~78us,
ACT ~60us, PE ~57us, SP ~22us of DMA triggers.
"""

import contextlib
import sys
import types
from contextlib import ExitStack

import numpy as np

import concourse.bass as bass
import concourse.tile as tile
from concourse import bacc, mybir
from concourse.bass_utils import run_bass_kernel_spmd


def _ensure_ntff_hook(so_path="/opt/axon/libaxon_pjrt.so"):
    """Provide antenv.axon_hooks (NTFF profiling hook) if the image lacks it."""
    try:
        from antenv.axon_hooks import get_axon_ntff_profile_hook  # noqa: F401

        return
    except ImportError:
        pass

    import ctypes

    mod = types.ModuleType("antenv.axon_hooks")
    mod._hook = None

    def set_axon_ntff_profile_hook(h):
        mod._hook = h

    def get_axon_ntff_profile_hook():
        return mod._hook

    mod.set_axon_ntff_profile_hook = set_axon_ntff_profile_hook
    mod.get_axon_ntff_profile_hook = get_axon_ntff_profile_hook
    sys.modules["antenv.axon_hooks"] = mod
    try:
        import antenv

        antenv.axon_hooks = mod
    except ImportError:
        pass

    try:
        lib = ctypes.CDLL(so_path)
    except OSError:
        return
    if not hasattr(lib, "axon_start_nrt_profile"):
        return
    lib.axon_start_nrt_profile.argtypes = [
        ctypes.POINTER(ctypes.c_int64),
        ctypes.c_size_t,
    ]
    lib.axon_start_nrt_profile.restype = ctypes.c_int64
    lib.axon_stop_nrt_profile.argtypes = [ctypes.c_char_p]
    lib.axon_stop_nrt_profile.restype = ctypes.c_int64

    @contextlib.contextmanager
    def _hook(output_dir, device_ids):
        import jax

        jax.devices()
        if device_ids:
            ids = (ctypes.c_int64 * len(device_ids))(*device_ids)
            rc = lib.axon_start_nrt_profile(ids, len(device_ids))
        else:
            rc = lib.axon_start_nrt_profile(None, 0)
        if rc != 0:
            raise RuntimeError(f"axon_start_nrt_profile rc={rc}")
        try:
            yield
        finally:
            n = lib.axon_stop_nrt_profile(str(output_dir).encode())
            if n <= 0:
                print(f"ntff profile capture wrote {n} files to {output_dir}")

    mod._hook = _hook


_ensure_ntff_hook()

M, N, D = 8192, 8192, 64
NCORES = 8
MS = M // NCORES  # 1024 rows per core
THRESH = 50.0

F32 = mybir.dt.float32
F16 = mybir.dt.float16
U16 = mybir.dt.uint16

ROWS_PER_SLAB = 128
GROUP = 4096          # detection-group columns (1 MB fp16 per DMA transfer)
CHUNK = 512           # PE/patch chunk (1 PSUM bank)
LOOKAHEAD = 4         # groups of input DMA issued ahead of compute


def build_kernel(ms=MS, n=N, d=D, num_devices=NCORES):
    """Build + compile the per-core SPMD program."""
    nc = bacc.Bacc(
        "TRN2",
        target_bir_lowering=False,
        debug=False,
        enable_asserts=False,
        num_devices=num_devices,
    )
    at_d = nc.dram_tensor("at", (d, n), F16, kind="ExternalInput")   # A.T
    bt_d = nc.dram_tensor("bt", (d, ms), F16, kind="ExternalInput")  # B_slab.T
    sg_d = nc.dram_tensor("sg", (d, n // GROUP), F16, kind="ExternalInput")
    c_d = nc.dram_tensor("c", (ms, n), F16, kind="ExternalInput")    # C slab
    out_d = nc.dram_tensor("out", (ms, n), F16, kind="ExternalOutput")

    nslabs = ms // ROWS_PER_SLAB          # 8
    ngroups = n // GROUP                  # 2
    nchunks = GROUP // CHUNK              # 8
    groups = [(r, g) for r in range(nslabs) for g in range(ngroups)]

    with tile.TileContext(nc) as tc, ExitStack() as ctx:
        consts = ctx.enter_context(tc.tile_pool(name="consts", bufs=1))
        cpool = ctx.enter_context(tc.tile_pool(name="cin", bufs=LOOKAHEAD + 4))
        opool = ctx.enter_context(tc.tile_pool(name="obuf", bufs=6))
        scpool = ctx.enter_context(tc.tile_pool(name="sc", bufs=4))
        mpool = ctx.enter_context(tc.tile_pool(name="m", bufs=4))
        ps_ct = ctx.enter_context(
            tc.tile_pool(name="ps_ct", bufs=4, space=bass.MemorySpace.PSUM)
        )

        # ---- one-time setup -------------------------------------------------
        bt_sb = consts.tile([d, ms], F16)
        sg_sb = consts.tile([d, ngroups], F16)
        at_sb = consts.tile([d, n], F16)

        nc.sync.dma_start(sg_sb[:], sg_d.ap())
        nc.sync.dma_start(bt_sb[:], bt_d.ap())

        ctiles = {}

        def prefetch(idx, eng=None):
            r, g = groups[idx]
            rows = slice(r * ROWS_PER_SLAB, (r + 1) * ROWS_PER_SLAB)
            gcols = slice(g * GROUP, (g + 1) * GROUP)
            ctile = cpool.tile([ROWS_PER_SLAB, GROUP], F16, name="cin", tag="cin")
            (eng or nc.sync).dma_start(ctile[:], c_d.ap()[rows, gcols])
            ctiles[idx] = ctile

        # interleave the big A.T load with the first C prefetches so the
        # first group's checksum path starts as soon as its MB lands.
        # Alternate ramp prefetches across the SP and scalar HW queues so
        # both DMA rings pull input during the ramp.
        prefetch(0)
        nc.sync.dma_start(at_sb[:, 0 : n // 2], at_d.ap()[:, 0 : n // 2])
        prefetch(1, nc.scalar)
        nc.sync.dma_start(at_sb[:, n // 2 : n], at_d.ap()[:, n // 2 : n])
        for i in range(2, LOOKAHEAD):
            prefetch(i, nc.scalar if i % 2 else None)

        # expected group sums for every slab, precomputed into SBUF during
        # the pipeline ramp: chk_sb[p, 2r+g] = B_slab[p] . s_g
        chk_sb = consts.tile([ROWS_PER_SLAB, nslabs * ngroups], F32)
        for r in range(nslabs):
            bt_r = bt_sb[:, r * ROWS_PER_SLAB : (r + 1) * ROWS_PER_SLAB]
            chk_ps = ps_ct.tile([ROWS_PER_SLAB, 2 * CHUNK], F32, tag="ct")
            nc.tensor.matmul(
                chk_ps[:, 0:ngroups], bt_r, sg_sb[:], start=True, stop=True
            )
            nc.scalar.activation(
                chk_sb[:, r * ngroups : (r + 1) * ngroups],
                chk_ps[:, 0:ngroups],
                mybir.ActivationFunctionType.Copy,
            )

        # ---- main streaming loop -------------------------------------------
        # out-DMA triggers go on the scalar engine (separate HW queue from the
        # SP input queue) and are delayed by one group so the scalar stream
        # never stalls waiting for the current group's patches. Once input
        # prefetching is done, the tail outputs alternate onto the now-idle
        # SP queue so both DMA rings drain the output stream.
        pending_out = None
        for idx, (r, g) in enumerate(groups):
            if idx + LOOKAHEAD < len(groups):
                prefetch(idx + LOOKAHEAD)
            rows = slice(r * ROWS_PER_SLAB, (r + 1) * ROWS_PER_SLAB)
            gcols = slice(g * GROUP, (g + 1) * GROUP)
            bt_r = bt_sb[:, r * ROWS_PER_SLAB : (r + 1) * ROWS_PER_SLAB]
            ctile = ctiles.pop(idx)

            # passthrough copy + fused per-row group sum of the faulty C
            obuf = opool.tile([ROWS_PER_SLAB, GROUP], F16, tag="ob")
            sc = scpool.tile([ROWS_PER_SLAB, 1], F32, tag="sc")
            nc.scalar.activation(
                obuf[:],
                ctile[:],
                mybir.ActivationFunctionType.Copy,
                accum_out=sc[:],
            )
            if pending_out is not None:
                pending_out()
                pending_out = None

            # row fault flags: m[p] = (chk - sc < -THRESH)  (faults add +100
            # per faulty element to sc, noise is < ~4)
            m = mpool.tile([ROWS_PER_SLAB, 1], U16, tag="m")
            nc.vector.tensor_scalar(
                m[:],
                chk_sb[:, r * ngroups + g : r * ngroups + g + 1],
                sc[:],
                -THRESH,
                mybir.AluOpType.subtract,
                mybir.AluOpType.is_lt,
            )

            # recompute C_true in 512-col matmuls into 1024-wide PSUM tiles,
            # patch flagged rows of obuf one pair of chunks at a time
            for h in range(nchunks // 2):
                ct_ps = ps_ct.tile([ROWS_PER_SLAB, 2 * CHUNK], F32, tag="ct")
                for j in range(2):
                    hh = 2 * h + j
                    cols = slice(
                        g * GROUP + hh * CHUNK, g * GROUP + (hh + 1) * CHUNK
                    )
                    nc.tensor.matmul(
                        ct_ps[:, j * CHUNK : (j + 1) * CHUNK],
                        bt_r,
                        at_sb[:, cols],
                        start=True,
                        stop=True,
                    )
                ocols = slice(2 * h * CHUNK, 2 * (h + 1) * CHUNK)
                nc.vector.copy_predicated(
                    obuf[:, ocols],
                    m[:].broadcast_to((ROWS_PER_SLAB, 2 * CHUNK)),
                    ct_ps[:],
                )

            out_eng = nc.scalar if idx % 2 else nc.sync
            out_args = (out_d.ap()[rows, gcols], obuf[:])
            pending_out = lambda e=out_eng, a=out_args: e.dma_start(*a)
        pending_out()

    nc.compile()
    return nc


def make_in_maps(A, B, C_faulty, ncores=NCORES, ms=MS):
    at = np.ascontiguousarray(A.T, dtype=np.float16)
    # per-group column sums of A.T == row-group sums of A (fp32 accum)
    sg = np.ascontiguousarray(
        A.astype(np.float32).reshape(N // GROUP, GROUP, D).sum(axis=1).T
    ).astype(np.float16)
    c16 = C_faulty.astype(np.float16)
    in_maps = []
    for i in range(ncores):
        rows = slice(i * ms, (i + 1) * ms)
        in_maps.append(
            {
                "at": at,
                "bt": np.ascontiguousarray(B[rows].T, dtype=np.float16),
                "sg": sg,
                "c": np.ascontiguousarray(c16[rows]),
            }
        )
    return in_maps


_NC_CACHE = {}


def kernel(A, B, C_faulty, **run_kwargs):
    A = np.asarray(A, dtype=np.float32)
    B = np.asarray(B, dtype=np.float32)
    C_faulty = np.asarray(C_faulty, dtype=np.float32)
    assert A.shape == (N, D) and B.shape == (M, D) and C_faulty.shape == (M, N)

    if "nc" not in _NC_CACHE:
        _NC_CACHE["nc"] = build_kernel()
    nc = _NC_CACHE["nc"]

    in_maps = make_in_maps(A, B, C_faulty)
    res = run_bass_kernel_spmd(nc, in_maps, core_ids=list(range(NCORES)), **run_kwargs)
    out = np.concatenate(
        [res.results[i]["out"].astype(np.float32) for i in range(NCORES)], axis=0
    )
    kernel.last_results = res
    return out


# revision 17
# speedup vs baseline: 1.0808x; 1.0808x over previous
"""Checksum-based fault detection + correction for C = B @ A.T on 8 trn2 cores.

Full inputs in, full output out. Rows of B / C_faulty are sharded across the
8 cores (data-parallel row slabs); A is replicated. C is streamed through the
device in fp16 (host casts during shard/gather; the harness gate is rel_err,
and fp16 round-trip costs ~2e-4 while halving HBM traffic).

Per 128-row slab x 4096-col group, each core:
  - streams the C group in (SP-triggered DMA, 1 MB per transfer),
  - ACT engine copies it to the output staging buffer obuf with a fused
    accumulate that yields the per-row group sum sc = sum_cols(C[p, group])
    in the same pass (detection coupling to C costs no extra engine pass),
  - PE computes the expected group sums chk = B_slab @ s_g (s_g = column
    sums of A.T per group, host-precomputed, one tiny 2-col matmul per slab)
    and recomputes ct = B_slab @ A.T chunk by chunk (8 x 512-col matmuls),
  - DVE forms the row flag m[p] = ((chk - sc) < -50) as uint16 in one small
    tensor_scalar op (faults shift a group sum by ~ +100 per faulty element;
    numerical noise is < ~4), then patches flagged rows of obuf from the
    recompute PSUM with one copy_predicated per 512-col chunk,
  - streams obuf back out (SP-triggered DMA).

Engine budget per core (16 groups): DMA ~92us (bound), DVE ~78us,
ACT ~60us, PE ~57us, SP ~22us of DMA triggers.
"""

import contextlib
import sys
import types
from contextlib import ExitStack

import numpy as np

import concourse.bass as bass
import concourse.tile as tile
from concourse import bacc, mybir
from concourse.bass_utils import run_bass_kernel_spmd


def _ensure_ntff_hook(so_path="/opt/axon/libaxon_pjrt.so"):
    """Provide antenv.axon_hooks (NTFF profiling hook) if the image lacks it."""
    try:
        from antenv.axon_hooks import get_axon_ntff_profile_hook  # noqa: F401

        return
    except ImportError:
        pass

    import ctypes

    mod = types.ModuleType("antenv.axon_hooks")
    mod._hook = None

    def set_axon_ntff_profile_hook(h):
        mod._hook = h

    def get_axon_ntff_profile_hook():
        return mod._hook

    mod.set_axon_ntff_profile_hook = set_axon_ntff_profile_hook
    mod.get_axon_ntff_profile_hook = get_axon_ntff_profile_hook
    sys.modules["antenv.axon_hooks"] = mod
    try:
        import antenv

        antenv.axon_hooks = mod
    except ImportError:
        pass

    try:
        lib = ctypes.CDLL(so_path)
    except OSError:
        return
    if not hasattr(lib, "axon_start_nrt_profile"):
        return
    lib.axon_start_nrt_profile.argtypes = [
        ctypes.POINTER(ctypes.c_int64),
        ctypes.c_size_t,
    ]
    lib.axon_start_nrt_profile.restype = ctypes.c_int64
    lib.axon_stop_nrt_profile.argtypes = [ctypes.c_char_p]
    lib.axon_stop_nrt_profile.restype = ctypes.c_int64

    @contextlib.contextmanager
    def _hook(output_dir, device_ids):
        import jax

        jax.devices()
        if device_ids:
            ids = (ctypes.c_int64 * len(device_ids))(*device_ids)
            rc = lib.axon_start_nrt_profile(ids, len(device_ids))
        else:
            rc = lib.axon_start_nrt_profile(None, 0)
        if rc != 0:
            raise RuntimeError(f"axon_start_nrt_profile rc={rc}")
        try:
            yield
        finally:
            n = lib.axon_stop_nrt_profile(str(output_dir).encode())
            if n <= 0:
                print(f"ntff profile capture wrote {n} files to {output_dir}")

    mod._hook = _hook


_ensure_ntff_hook()

M, N, D = 8192, 8192, 64
NCORES = 8
MS = M // NCORES  # 1024 rows per core
THRESH = 50.0

F32 = mybir.dt.float32
F16 = mybir.dt.float16
U16 = mybir.dt.uint16

ROWS_PER_SLAB = 128
GROUP = 4096          # detection-group columns (1 MB fp16 per DMA transfer)
CHUNK = 512           # PE/patch chunk (1 PSUM bank)
LOOKAHEAD = 4         # groups of input DMA issued ahead of compute


def build_kernel(ms=MS, n=N, d=D, num_devices=NCORES):
    """Build + compile the per-core SPMD program."""
    nc = bacc.Bacc(
        "TRN2",
        target_bir_lowering=False,
        debug=False,
        enable_asserts=False,
        num_devices=num_devices,
    )
    at_d = nc.dram_tensor("at", (d, n), F16, kind="ExternalInput")   # A.T
    bt_d = nc.dram_tensor("bt", (d, ms), F16, kind="ExternalInput")  # B_slab.T
    sg_d = nc.dram_tensor("sg", (d, n // GROUP), F16, kind="ExternalInput")
    c_d = nc.dram_tensor("c", (ms, n), F16, kind="ExternalInput")    # C slab
    out_d = nc.dram_tensor("out", (ms, n), F16, kind="ExternalOutput")

    nslabs = ms // ROWS_PER_SLAB          # 8
    ngroups = n // GROUP                  # 2
    nchunks = GROUP // CHUNK              # 8
    groups = [(r, g) for r in range(nslabs) for g in range(ngroups)]

    with tile.TileContext(nc) as tc, ExitStack() as ctx:
        consts = ctx.enter_context(tc.tile_pool(name="consts", bufs=1))
        cpool = ctx.enter_context(tc.tile_pool(name="cin", bufs=LOOKAHEAD + 4))
        opool = ctx.enter_context(tc.tile_pool(name="obuf", bufs=6))
        scpool = ctx.enter_context(tc.tile_pool(name="sc", bufs=4))
        mpool = ctx.enter_context(tc.tile_pool(name="m", bufs=4))
        ps_ct = ctx.enter_context(
            tc.tile_pool(name="ps_ct", bufs=4, space=bass.MemorySpace.PSUM)
        )

        # ---- one-time setup -------------------------------------------------
        bt_sb = consts.tile([d, ms], F16)
        sg_sb = consts.tile([d, ngroups], F16)
        at_sb = consts.tile([d, n], F16)

        nc.sync.dma_start(sg_sb[:], sg_d.ap())
        nc.sync.dma_start(bt_sb[:], bt_d.ap())

        ctiles = {}

        def prefetch(idx, eng=None):
            r, g = groups[idx]
            rows = slice(r * ROWS_PER_SLAB, (r + 1) * ROWS_PER_SLAB)
            gcols = slice(g * GROUP, (g + 1) * GROUP)
            ctile = cpool.tile([ROWS_PER_SLAB, GROUP], F16, name="cin", tag="cin")
            (eng or nc.sync).dma_start(ctile[:], c_d.ap()[rows, gcols])
            ctiles[idx] = ctile

        # interleave the big A.T load with the first C prefetches so the
        # first group's checksum path starts as soon as its MB lands.
        # Alternate ramp prefetches across the SP and scalar HW queues so
        # both DMA rings pull input during the ramp.
        prefetch(0)
        nc.sync.dma_start(at_sb[:, 0 : n // 2], at_d.ap()[:, 0 : n // 2])
        prefetch(1, nc.scalar)
        nc.sync.dma_start(at_sb[:, n // 2 : n], at_d.ap()[:, n // 2 : n])
        for i in range(2, LOOKAHEAD):
            prefetch(i, nc.scalar if i % 2 else None)

        # expected group sums for every slab, precomputed into SBUF during
        # the pipeline ramp: chk_sb[p, 2r+g] = B_slab[p] . s_g
        chk_sb = consts.tile([ROWS_PER_SLAB, nslabs * ngroups], F32)
        for r in range(nslabs):
            bt_r = bt_sb[:, r * ROWS_PER_SLAB : (r + 1) * ROWS_PER_SLAB]
            chk_ps = ps_ct.tile([ROWS_PER_SLAB, 2 * CHUNK], F32, tag="ct")
            nc.tensor.matmul(
                chk_ps[:, 0:ngroups], bt_r, sg_sb[:], start=True, stop=True
            )
            nc.scalar.activation(
                chk_sb[:, r * ngroups : (r + 1) * ngroups],
                chk_ps[:, 0:ngroups],
                mybir.ActivationFunctionType.Copy,
            )

        # ---- main streaming loop -------------------------------------------
        # out-DMA triggers go on the scalar engine (separate HW queue from the
        # SP input queue) and are delayed by one group so the scalar stream
        # never stalls waiting for the current group's patches. Once input
        # prefetching is done, the tail outputs alternate onto the now-idle
        # SP queue so both DMA rings drain the output stream.
        pending_out = None
        for idx, (r, g) in enumerate(groups):
            if idx + LOOKAHEAD < len(groups):
                prefetch(idx + LOOKAHEAD)
            rows = slice(r * ROWS_PER_SLAB, (r + 1) * ROWS_PER_SLAB)
            gcols = slice(g * GROUP, (g + 1) * GROUP)
            bt_r = bt_sb[:, r * ROWS_PER_SLAB : (r + 1) * ROWS_PER_SLAB]
            ctile = ctiles.pop(idx)

            # passthrough copy + fused per-row group sum of the faulty C
            obuf = opool.tile([ROWS_PER_SLAB, GROUP], F16, tag="ob")
            sc = scpool.tile([ROWS_PER_SLAB, 1], F32, tag="sc")
            nc.scalar.activation(
                obuf[:],
                ctile[:],
                mybir.ActivationFunctionType.Copy,
                accum_out=sc[:],
            )
            if pending_out is not None:
                pending_out()
                pending_out = None

            # row fault flags: m[p] = (chk - sc < -THRESH)  (faults add +100
            # per faulty element to sc, noise is < ~4); runs on the otherwise
            # idle GPSIMD engine to keep DVE free for patching
            m = mpool.tile([ROWS_PER_SLAB, 1], U16, tag="m")
            nc.gpsimd.tensor_scalar(
                m[:],
                chk_sb[:, r * ngroups + g : r * ngroups + g + 1],
                sc[:],
                -THRESH,
                mybir.AluOpType.subtract,
                mybir.AluOpType.is_lt,
            )

            # recompute C_true in 512-col matmuls into 1024-wide PSUM tiles,
            # patch flagged rows of obuf one pair of chunks at a time
            for h in range(nchunks // 2):
                ct_ps = ps_ct.tile([ROWS_PER_SLAB, 2 * CHUNK], F32, tag="ct")
                for j in range(2):
                    hh = 2 * h + j
                    cols = slice(
                        g * GROUP + hh * CHUNK, g * GROUP + (hh + 1) * CHUNK
                    )
                    nc.tensor.matmul(
                        ct_ps[:, j * CHUNK : (j + 1) * CHUNK],
                        bt_r,
                        at_sb[:, cols],
                        start=True,
                        stop=True,
                    )
                ocols = slice(2 * h * CHUNK, 2 * (h + 1) * CHUNK)
                nc.vector.copy_predicated(
                    obuf[:, ocols],
                    m[:].broadcast_to((ROWS_PER_SLAB, 2 * CHUNK)),
                    ct_ps[:],
                )

            # out-triggers ride the idle GPSIMD engine (SWDGE) so neither
            # the ACT copy stream nor the SP input stream ever stalls on a
            # patch-completion wait
            out_args = (out_d.ap()[rows, gcols], obuf[:])
            pending_out = lambda a=out_args: nc.gpsimd.dma_start(*a)
        pending_out()

    nc.compile()
    return nc


def make_in_maps(A, B, C_faulty, ncores=NCORES, ms=MS):
    at = np.ascontiguousarray(A.T, dtype=np.float16)
    # per-group column sums of A.T == row-group sums of A (fp32 accum)
    sg = np.ascontiguousarray(
        A.astype(np.float32).reshape(N // GROUP, GROUP, D).sum(axis=1).T
    ).astype(np.float16)
    c16 = C_faulty.astype(np.float16)
    in_maps = []
    for i in range(ncores):
        rows = slice(i * ms, (i + 1) * ms)
        in_maps.append(
            {
                "at": at,
                "bt": np.ascontiguousarray(B[rows].T, dtype=np.float16),
                "sg": sg,
                "c": np.ascontiguousarray(c16[rows]),
            }
        )
    return in_maps


_NC_CACHE = {}


def kernel(A, B, C_faulty, **run_kwargs):
    A = np.asarray(A, dtype=np.float32)
    B = np.asarray(B, dtype=np.float32)
    C_faulty = np.asarray(C_faulty, dtype=np.float32)
    assert A.shape == (N, D) and B.shape == (M, D) and C_faulty.shape == (M, N)

    if "nc" not in _NC_CACHE:
        _NC_CACHE["nc"] = build_kernel()
    nc = _NC_CACHE["nc"]

    in_maps = make_in_maps(A, B, C_faulty)
    res = run_bass_kernel_spmd(nc, in_maps, core_ids=list(range(NCORES)), **run_kwargs)
    out = np.concatenate(
        [res.results[i]["out"].astype(np.float32) for i in range(NCORES)], axis=0
    )
    kernel.last_results = res
    return out


# revision 18
# speedup vs baseline: 1.1656x; 1.0785x over previous
"""Checksum-based fault detection + correction for C = B @ A.T on 8 trn2 cores.

Full inputs in, full output out. Rows of B / C_faulty are sharded across the
8 cores (data-parallel row slabs); A is replicated. C is streamed through the
device in fp16 (host casts during shard/gather; the harness gate is rel_err,
and fp16 round-trip costs ~2e-4 while halving HBM traffic).

Per 128-row slab x 4096-col group, each core:
  - streams the C group in (SP-triggered DMA, 1 MB per transfer),
  - ACT engine copies it to the output staging buffer obuf with a fused
    accumulate that yields the per-row group sum sc = sum_cols(C[p, group])
    in the same pass (detection coupling to C costs no extra engine pass),
  - PE computes the expected group sums chk = B_slab @ s_g (s_g = column
    sums of A.T per group, host-precomputed, one tiny 2-col matmul per slab)
    and recomputes ct = B_slab @ A.T chunk by chunk (8 x 512-col matmuls),
  - DVE forms the row flag m[p] = ((chk - sc) < -50) as uint16 in one small
    tensor_scalar op (faults shift a group sum by ~ +100 per faulty element;
    numerical noise is < ~4), then patches flagged rows of obuf from the
    recompute PSUM with one copy_predicated per 512-col chunk,
  - streams obuf back out (SP-triggered DMA).

Engine budget per core (16 groups): DMA ~92us (bound), DVE ~78us,
ACT ~60us, PE ~57us, SP ~22us of DMA triggers.
"""

import contextlib
import sys
import types
from contextlib import ExitStack

import numpy as np

import concourse.bass as bass
import concourse.tile as tile
from concourse import bacc, mybir
from concourse.bass_utils import run_bass_kernel_spmd


def _ensure_ntff_hook(so_path="/opt/axon/libaxon_pjrt.so"):
    """Provide antenv.axon_hooks (NTFF profiling hook) if the image lacks it."""
    try:
        from antenv.axon_hooks import get_axon_ntff_profile_hook  # noqa: F401

        return
    except ImportError:
        pass

    import ctypes

    mod = types.ModuleType("antenv.axon_hooks")
    mod._hook = None

    def set_axon_ntff_profile_hook(h):
        mod._hook = h

    def get_axon_ntff_profile_hook():
        return mod._hook

    mod.set_axon_ntff_profile_hook = set_axon_ntff_profile_hook
    mod.get_axon_ntff_profile_hook = get_axon_ntff_profile_hook
    sys.modules["antenv.axon_hooks"] = mod
    try:
        import antenv

        antenv.axon_hooks = mod
    except ImportError:
        pass

    try:
        lib = ctypes.CDLL(so_path)
    except OSError:
        return
    if not hasattr(lib, "axon_start_nrt_profile"):
        return
    lib.axon_start_nrt_profile.argtypes = [
        ctypes.POINTER(ctypes.c_int64),
        ctypes.c_size_t,
    ]
    lib.axon_start_nrt_profile.restype = ctypes.c_int64
    lib.axon_stop_nrt_profile.argtypes = [ctypes.c_char_p]
    lib.axon_stop_nrt_profile.restype = ctypes.c_int64

    @contextlib.contextmanager
    def _hook(output_dir, device_ids):
        import jax

        jax.devices()
        if device_ids:
            ids = (ctypes.c_int64 * len(device_ids))(*device_ids)
            rc = lib.axon_start_nrt_profile(ids, len(device_ids))
        else:
            rc = lib.axon_start_nrt_profile(None, 0)
        if rc != 0:
            raise RuntimeError(f"axon_start_nrt_profile rc={rc}")
        try:
            yield
        finally:
            n = lib.axon_stop_nrt_profile(str(output_dir).encode())
            if n <= 0:
                print(f"ntff profile capture wrote {n} files to {output_dir}")

    mod._hook = _hook


_ensure_ntff_hook()

M, N, D = 8192, 8192, 64
NCORES = 8
MS = M // NCORES  # 1024 rows per core
THRESH = 50.0

F32 = mybir.dt.float32
F16 = mybir.dt.float16
U16 = mybir.dt.uint16

ROWS_PER_SLAB = 128
GROUP = 4096          # detection-group columns (1 MB fp16 per DMA transfer)
CHUNK = 512           # PE/patch chunk (1 PSUM bank)
LOOKAHEAD = 4         # groups of input DMA issued ahead of compute


def build_kernel(ms=MS, n=N, d=D, num_devices=NCORES):
    """Build + compile the per-core SPMD program."""
    nc = bacc.Bacc(
        "TRN2",
        target_bir_lowering=False,
        debug=False,
        enable_asserts=False,
        num_devices=num_devices,
    )
    at_d = nc.dram_tensor("at", (d, n), F16, kind="ExternalInput")   # A.T
    bt_d = nc.dram_tensor("bt", (d, ms), F16, kind="ExternalInput")  # B_slab.T
    sg_d = nc.dram_tensor("sg", (d, n // GROUP), F16, kind="ExternalInput")
    c_d = nc.dram_tensor("c", (ms, n), F16, kind="ExternalInput")    # C slab
    out_d = nc.dram_tensor("out", (ms, n), F16, kind="ExternalOutput")

    nslabs = ms // ROWS_PER_SLAB          # 8
    ngroups = n // GROUP                  # 2
    nchunks = GROUP // CHUNK              # 8
    groups = [(r, g) for r in range(nslabs) for g in range(ngroups)]

    with tile.TileContext(nc) as tc, ExitStack() as ctx:
        consts = ctx.enter_context(tc.tile_pool(name="consts", bufs=1))
        cpool = ctx.enter_context(tc.tile_pool(name="cin", bufs=LOOKAHEAD + 4))
        opool = ctx.enter_context(tc.tile_pool(name="obuf", bufs=6))
        scpool = ctx.enter_context(tc.tile_pool(name="sc", bufs=4))
        mpool = ctx.enter_context(tc.tile_pool(name="m", bufs=4))
        ps_ct = ctx.enter_context(
            tc.tile_pool(name="ps_ct", bufs=4, space=bass.MemorySpace.PSUM)
        )

        # ---- one-time setup -------------------------------------------------
        bt_sb = consts.tile([d, ms], F16)
        sg_sb = consts.tile([d, ngroups], F16)
        at_sb = consts.tile([d, n], F16)

        nc.sync.dma_start(sg_sb[:], sg_d.ap())
        nc.sync.dma_start(bt_sb[:], bt_d.ap())

        ctiles = {}

        def prefetch(idx, eng=None):
            r, g = groups[idx]
            rows = slice(r * ROWS_PER_SLAB, (r + 1) * ROWS_PER_SLAB)
            gcols = slice(g * GROUP, (g + 1) * GROUP)
            ctile = cpool.tile([ROWS_PER_SLAB, GROUP], F16, name="cin", tag="cin")
            (eng or nc.sync).dma_start(ctile[:], c_d.ap()[rows, gcols])
            ctiles[idx] = ctile

        # interleave the big A.T load with the first C prefetches so the
        # first group's checksum path starts as soon as its MB lands.
        # Alternate ramp prefetches across the SP and scalar HW queues so
        # both DMA rings pull input during the ramp.
        prefetch(0)
        nc.sync.dma_start(at_sb[:, 0 : n // 2], at_d.ap()[:, 0 : n // 2])
        prefetch(1, nc.scalar)
        nc.sync.dma_start(at_sb[:, n // 2 : n], at_d.ap()[:, n // 2 : n])
        for i in range(2, LOOKAHEAD):
            prefetch(i, nc.scalar if i % 2 else None)
        RAMP = 8  # prefetches below this index alternate onto the ACT queue

        # expected group sums for every slab, precomputed into SBUF during
        # the pipeline ramp: chk_sb[p, 2r+g] = B_slab[p] . s_g
        chk_sb = consts.tile([ROWS_PER_SLAB, nslabs * ngroups], F32)
        for r in range(nslabs):
            bt_r = bt_sb[:, r * ROWS_PER_SLAB : (r + 1) * ROWS_PER_SLAB]
            chk_ps = ps_ct.tile([ROWS_PER_SLAB, 2 * CHUNK], F32, tag="ct")
            nc.tensor.matmul(
                chk_ps[:, 0:ngroups], bt_r, sg_sb[:], start=True, stop=True
            )
            nc.scalar.activation(
                chk_sb[:, r * ngroups : (r + 1) * ngroups],
                chk_ps[:, 0:ngroups],
                mybir.ActivationFunctionType.Copy,
            )

        # ---- main streaming loop -------------------------------------------
        # out-DMA triggers go on the scalar engine (separate HW queue from the
        # SP input queue) and are delayed by one group so the scalar stream
        # never stalls waiting for the current group's patches. Once input
        # prefetching is done, the tail outputs alternate onto the now-idle
        # SP queue so both DMA rings drain the output stream.
        pending_out = None
        for idx, (r, g) in enumerate(groups):
            if idx + LOOKAHEAD < len(groups):
                j = idx + LOOKAHEAD
                prefetch(j, nc.scalar if (j < RAMP and j % 2) else None)
            rows = slice(r * ROWS_PER_SLAB, (r + 1) * ROWS_PER_SLAB)
            gcols = slice(g * GROUP, (g + 1) * GROUP)
            bt_r = bt_sb[:, r * ROWS_PER_SLAB : (r + 1) * ROWS_PER_SLAB]
            ctile = ctiles.pop(idx)

            # passthrough copy + fused per-row group sum of the faulty C
            obuf = opool.tile([ROWS_PER_SLAB, GROUP], F16, tag="ob")
            sc = scpool.tile([ROWS_PER_SLAB, 1], F32, tag="sc")
            nc.scalar.activation(
                obuf[:],
                ctile[:],
                mybir.ActivationFunctionType.Copy,
                accum_out=sc[:],
            )
            if pending_out is not None:
                pending_out()
                pending_out = None

            # row fault flags: m[p] = (chk - sc < -THRESH)  (faults add +100
            # per faulty element to sc, noise is < ~4); runs on the otherwise
            # idle GPSIMD engine to keep DVE free for patching
            m = mpool.tile([ROWS_PER_SLAB, 1], U16, tag="m")
            nc.gpsimd.tensor_scalar(
                m[:],
                chk_sb[:, r * ngroups + g : r * ngroups + g + 1],
                sc[:],
                -THRESH,
                mybir.AluOpType.subtract,
                mybir.AluOpType.is_lt,
            )

            # recompute C_true in 512-col matmuls into 1024-wide PSUM tiles,
            # patch flagged rows of obuf one pair of chunks at a time
            for h in range(nchunks // 2):
                ct_ps = ps_ct.tile([ROWS_PER_SLAB, 2 * CHUNK], F32, tag="ct")
                for j in range(2):
                    hh = 2 * h + j
                    cols = slice(
                        g * GROUP + hh * CHUNK, g * GROUP + (hh + 1) * CHUNK
                    )
                    nc.tensor.matmul(
                        ct_ps[:, j * CHUNK : (j + 1) * CHUNK],
                        bt_r,
                        at_sb[:, cols],
                        start=True,
                        stop=True,
                    )
                ocols = slice(2 * h * CHUNK, 2 * (h + 1) * CHUNK)
                nc.vector.copy_predicated(
                    obuf[:, ocols],
                    m[:].broadcast_to((ROWS_PER_SLAB, 2 * CHUNK)),
                    ct_ps[:],
                )

            # out-triggers ride the idle GPSIMD engine (SWDGE) so neither
            # the ACT copy stream nor the SP input stream ever stalls on a
            # patch-completion wait; once input prefetching is over, tail
            # outputs alternate onto the idle SP queue to drain two-wide
            out_eng = nc.sync if (idx + LOOKAHEAD >= len(groups) and idx % 2) else nc.gpsimd
            out_args = (out_d.ap()[rows, gcols], obuf[:])
            pending_out = lambda e=out_eng, a=out_args: e.dma_start(*a)
        pending_out()

    nc.compile()
    return nc


def make_in_maps(A, B, C_faulty, ncores=NCORES, ms=MS):
    at = np.ascontiguousarray(A.T, dtype=np.float16)
    # per-group column sums of A.T == row-group sums of A (fp32 accum)
    sg = np.ascontiguousarray(
        A.astype(np.float32).reshape(N // GROUP, GROUP, D).sum(axis=1).T
    ).astype(np.float16)
    c16 = C_faulty.astype(np.float16)
    in_maps = []
    for i in range(ncores):
        rows = slice(i * ms, (i + 1) * ms)
        in_maps.append(
            {
                "at": at,
                "bt": np.ascontiguousarray(B[rows].T, dtype=np.float16),
                "sg": sg,
                "c": np.ascontiguousarray(c16[rows]),
            }
        )
    return in_maps


_NC_CACHE = {}


def kernel(A, B, C_faulty, **run_kwargs):
    A = np.asarray(A, dtype=np.float32)
    B = np.asarray(B, dtype=np.float32)
    C_faulty = np.asarray(C_faulty, dtype=np.float32)
    assert A.shape == (N, D) and B.shape == (M, D) and C_faulty.shape == (M, N)

    if "nc" not in _NC_CACHE:
        _NC_CACHE["nc"] = build_kernel()
    nc = _NC_CACHE["nc"]

    in_maps = make_in_maps(A, B, C_faulty)
    res = run_bass_kernel_spmd(nc, in_maps, core_ids=list(range(NCORES)), **run_kwargs)
    out = np.concatenate(
        [res.results[i]["out"].astype(np.float32) for i in range(NCORES)], axis=0
    )
    kernel.last_results = res
    return out
